# revision 73
# baseline (speedup 1.0000x reference)
"""Trainium2 Bass kernel for nn_Attention_80384607912675.

Multi-head attention (B=2, S=2048, D=1024, H=16, HD=64), fp32 reference.

Sharding (8 cores): data-parallel over batch (2) x tensor-parallel over heads
(4 head groups of 4 heads).  Core c handles batch c//4, heads [4*(c%4), 4*(c%4)+4).
wq/wk/wv split column-wise, wo split row-wise; the wo partial sums are reduced
on the host.

Per-core kernel (all matmuls bf16 with fp32 PSUM accumulation):
  QT/KT = (x @ wq/k)^T  stored head-major [256 -> 2x128, 2048]
  V_aug = [x @ wv | 1]  stored NATURAL [2048, 4*(64+1)] (no transposes: the
                        projection emits [s,d] tiles directly; ones column per
                        head folds the softmax row-sum into PV)
  per window w = (qw, hp) (q-window 512 wide, head pair hp):
    S^T[kp, q] = K_h^T (x) Q_h  (A,B packed in one [128,1024] PSUM tile)
    P^T        = exp(S^T / 8)   (one ScalarE instr per A|B pair, ->bf16)
    O[q, d]   += P^T_slice^T (x) V_aug  (O NATURAL: out [128q, 65] regions,
                128-partition output, ~2x cheaper on PE than the O^T form)
  normalization is per-PARTITION (rowsum col 64 of each region): one DVE
  tensor_scalar (mult by reciprocal AP) per region -> bf16, then PE-transposed
  into O^T (onm2) for the output projection (head B lands at PSUM base 64 via
  tile_position so no partition-relocation DMA is needed).
  outproj: ONE fused [2048,1024] partial per core (both pair blocks
  accumulated in PSUM), DMA'd bf16; host adds 4 partials per batch + bo.

PE is the bottleneck (~140us busy); everything else (exp stream on ACT ~133us,
copies/norm on DVE, DMA) hides under it except the serial input-DMA lead-in.
"""

import numpy as np

B, S, D, H = 2, 2048, 1024, 16
HD = D // H          # 64
HPC = 4              # heads per core
DHC = HPC * HD       # 256 head dims per core
KC = D // 128        # 8 contraction chunks
SB = S // 128        # 16 s blocks / kp chunks
VP = HPC * (HD + 1)  # 260: V storage pitch per s-chunk (ones col per head)
NC = 8               # cores
NQW = 4              # 512-wide q windows per head pair
NW = 8               # windows: w -> (qw = w//2, hp = w%2)

_nc_cache = {}


def _build_bass(with_bias=False, debug=False):
    import concourse.mybir as mybir
    import concourse.tile as tile
    from concourse import bacc

    BF = mybir.dt.bfloat16
    F32 = mybir.dt.float32
    EXP = mybir.ActivationFunctionType.Exp

    nc = bacc.Bacc("TRN2")

    xT_d = nc.dram_tensor("xT", [D, S], BF, kind="ExternalInput")
    wq_d = nc.dram_tensor("wq_c", [D, DHC], BF, kind="ExternalInput")
    wk_d = nc.dram_tensor("wk_c", [D, DHC], BF, kind="ExternalInput")
    wv_d = nc.dram_tensor("wv_c", [D, DHC], BF, kind="ExternalInput")
    wo_d = nc.dram_tensor("wo_c", [DHC, D], BF, kind="ExternalInput")
    bias_d = nc.dram_tensor("bias3", [1, 3 * DHC], BF, kind="ExternalInput")
    out_d = nc.dram_tensor("out", [S, D], BF, kind="ExternalOutput")
    if debug:
        dbg = {
            "qt": nc.dram_tensor("dbg_qt", [128, 2 * S], BF, kind="ExternalOutput"),
            "kt": nc.dram_tensor("dbg_kt", [128, 2 * S], BF, kind="ExternalOutput"),
            "v": nc.dram_tensor("dbg_v", [128, SB * VP], BF, kind="ExternalOutput"),
            "onm2": nc.dram_tensor(
                "dbg_onm2", [128, 2 * S], BF, kind="ExternalOutput"
            ),
        }

    with tile.TileContext(nc) as tc:
        with (
            tc.tile_pool(name="persist", bufs=1) as pp,
            tc.tile_pool(name="sc", bufs=2, space="PSUM") as scp,
            tc.tile_pool(name="oacc", bufs=1, space="PSUM") as opp,
            tc.tile_pool(name="pj", bufs=1, space="PSUM") as pjp,
            tc.tile_pool(name="po", bufs=1, space="PSUM") as pop,
            tc.tile_pool(name="pt", bufs=16) as ptp,
            tc.tile_pool(name="onorm", bufs=3) as onp,
            tc.tile_pool(name="rc", bufs=4) as rcp,
            tc.tile_pool(name="osb", bufs=8) as oup,
        ):
            # PSUM budget (16KB/partition = 8 banks): sc 2x[128,1024]f32 +
            # oacc [128,1024]f32 + pj [128,512]f32 (projection accum) +
            # po [128,512]f32 (outproj accum / transpose scratch). The PE
            # runs in program order, so filler work is dripped in <=2-matmul
            # bites per slot to keep each slot's PE time under the 1038ns
            # ACT exp that paces the steady-state window.
            xT_sb = pp.tile([128, KC * S], BF, tag="xT", name="xT_sb")
            wq_sb = pp.tile([128, KC * DHC], BF, tag="wq", name="wq_sb")
            wk_sb = pp.tile([128, KC * DHC], BF, tag="wk", name="wk_sb")
            wv_sb = pp.tile([128, KC * DHC], BF, tag="wv", name="wv_sb")
            wo_sb = pp.tile([128, 2 * D], BF, tag="wo", name="wo_sb")
            qt_sb = pp.tile([128, 2 * S], BF, tag="qt", name="qt_sb")
            kt_sb = pp.tile([128, 2 * S], BF, tag="kt", name="kt_sb")
            v_sb = pp.tile([128, SB * VP], BF, tag="v", name="v_sb")
            onm2_sb = pp.tile([128, 2 * S], BF, tag="onm2", name="onm2_sb")
            ident = pp.tile([128, 128], BF, tag="ident", name="ident")
            bias_sb = pp.tile([1, 3 * DHC], BF, tag="bias", name="bias_sb")
            ones16 = pp.tile([1, 512], BF, tag="ones16", name="ones16")

            # input DMAs (DMA is serial: order = arrival order). Weights
            # needed by the k-major lead-in come first, then the xT chunks
            # consumed per k; wo last (needed late).
            def load_w(w_sb, w_d):
                nc.sync.dma_start(
                    w_sb[:, :].rearrange("p (k d) -> p k d", d=DHC),
                    w_d[:, :].rearrange("(k p) d -> p k d", p=128),
                )

            def load_xt(k):
                nc.sync.dma_start(
                    xT_sb[:, k * S:(k + 1) * S], xT_d[k * 128:(k + 1) * 128, :]
                )

            def load_xt_half(k, h):
                nc.sync.dma_start(
                    xT_sb[:, k * S + h * 1024:(k * S) + (h + 1) * 1024],
                    xT_d[k * 128:(k + 1) * 128, h * 1024:(h + 1) * 1024],
                )

            load_w(wk_sb, wk_d)
            load_w(wq_sb, wq_d)
            load_xt(0)
            load_w(wv_sb, wv_d)
            for k in range(1, KC - 1):
                load_xt(k)
            # split the last chunk so the lead's final k-iteration (nt0/nt1/
            # qt/V need only cols 0-1023) starts a half-chunk earlier
            load_xt_half(KC - 1, 0)
            load_xt_half(KC - 1, 1)
            nc.sync.dma_start(bias_sb[:, :], bias_d[:, :])
            nc.sync.dma_start(
                wo_sb[:, :].rearrange("r (p d) -> r p d", d=D),
                wo_d[:, :].rearrange("(p r) d -> r p d", r=128),
            )
            zt = pp.tile([128, 512], BF, tag="zt", name="zt")
            nc.vector.memset(ones16[:, :], 1.0)
            nc.vector.memset(zt[:, :], 0.0)
            # ones columns of V_aug: preset everything to 1, V overwrites below
            nc.gpsimd.memset(v_sb[:, :], 1.0)
            from concourse.masks import make_identity
            make_identity(nc, ident[:, :])

            bq = bias_sb[0:1, 0:DHC]
            bk = bias_sb[0:1, DHC:2 * DHC]
            bv = bias_sb[0:1, 2 * DHC:3 * DHC]

            # ---------------- projection helpers ----------------
            def qk_mm(ps, w_sb, p, nt, k):
                nc.tensor.matmul(
                    ps[:, :],
                    lhsT=w_sb[:, k * DHC + p * 128: k * DHC + (p + 1) * 128],
                    rhs=xT_sb[:, k * S + nt * 512: k * S + (nt + 1) * 512],
                    start=(k == 0),
                    stop=(k == KC - 1 and not with_bias),
                )

            def qk_fin(ps, dst, bias, p, nt, on_act=False):
                if with_bias:
                    nc.tensor.matmul(
                        ps[:, :],
                        lhsT=bias[:, p * 128:(p + 1) * 128],
                        rhs=ones16[0:1, :],
                        start=False,
                        stop=True,
                    )
                dslice = dst[:, p * S + nt * 512: p * S + (nt + 1) * 512]
                if on_act:
                    nc.scalar.copy(dslice, ps[:, :])
                else:
                    nc.vector.tensor_copy(dslice, ps[:, :])

            def qk_chunk(box, w_sb, bias, p, nt, ks, pool=None):
                if 0 in ks:
                    pool = pool or pjp
                    box["ps"] = pool.tile(
                        [128, 512], F32, tag=pool.name, name=f"qk_{p}_{nt}"
                    )
                for k in ks:
                    qk_mm(box["ps"], w_sb, p, nt, k)

            def v_mm(ps, c, k, ap=None, no_start=False):
                nc.tensor.matmul(
                    ps if ap is None else ap,
                    lhsT=xT_sb[:, k * S + c * 128: k * S + (c + 1) * 128],
                    rhs=wv_sb[:, k * DHC:(k + 1) * DHC],
                    start=(k == 0 and not no_start),
                    stop=(k == KC - 1 and not with_bias),
                    skip_group_check=no_start,
                )

            def v_fin(ps_ap, c, eng=None):
                if with_bias:
                    nc.tensor.matmul(
                        ps_ap,
                        lhsT=ones16[0:1, 0:128],
                        rhs=bv[:, :],
                        start=False,
                        stop=True,
                    )
                dst3 = v_sb[
                    :, c * VP:(c + 1) * VP
                ].rearrange("p (h e) -> p h e", e=HD + 1)[:, :, 0:HD]
                if eng is nc.scalar:
                    nc.scalar.copy(dst3, ps_ap)
                else:
                    (eng or nc.vector).tensor_copy(dst3, ps_ap)

            def proj_v(c, eng=None, pool=None):
                """V s-chunk c: [128 s, 256 d] natural, all K chunks, + fin."""
                pool = pool or pjp
                ps = pool.tile([128, 512], F32, tag=pool.name, name=f"v_{c}")
                ap = ps[:, 0:DHC]
                for k in range(KC):
                    v_mm(None, c, k, ap=ap)
                v_fin(ap, c, eng=eng)

            # ---------------- outproj ----------------
            def outproj_mm(po, sb, n, p):
                nc.tensor.matmul(
                    po[:, :],
                    lhsT=onm2_sb[:, p * S + sb * 128: p * S + (sb + 1) * 128],
                    rhs=wo_sb[:, p * D + n * 512: p * D + (n + 1) * 512],
                    start=(p == 0),
                    stop=(p == 1),
                )

            _ot_cache = {}

            def outproj_fin(po, sb, n, on_act=False, eng=None):
                # both n-halves of an sb share one SBUF tile and one DMA:
                # halves the HWDGE count so the tail's DMA stream drains fast
                if sb not in _ot_cache:
                    _ot_cache[sb] = oup.tile(
                        [128, 1024], BF, tag="osb", name=f"ot_{sb}"
                    )
                ot = _ot_cache[sb]
                if on_act:
                    nc.scalar.copy(ot[:, n * 512:(n + 1) * 512], po[:, :])
                else:
                    (eng or nc.vector).tensor_copy(
                        ot[:, n * 512:(n + 1) * 512], po[:, :]
                    )
                if sb in _ot_done:
                    del _ot_cache[sb]
                    nc.sync.dma_start(
                        out_d[sb * 128:(sb + 1) * 128, :], ot[:, :]
                    )
                else:
                    _ot_done.add(sb)

            _ot_done = set()

            def outproj_piece(sb, n, on_act=False, po=None, eng=None):
                if po is None:
                    po = pop.tile([128, 512], F32, tag="po", name=f"po_{sb}_{n}")
                for p in range(2):
                    outproj_mm(po, sb, n, p)
                outproj_fin(po, sb, n, on_act, eng)

            # ---------------- drain (normalize + transpose) ----------------
            def drain_sums(w, oacc):
                """rowsum cols -> reciprocal (one DVE op for all 8 regions)."""
                rs = rcp.tile([128, 8], F32, tag="rc", name=f"rs_{w}")
                rc = rcp.tile([128, 8], F32, tag="rc", name=f"rc_{w}")
                sums = oacc[:, :].rearrange("p (r e) -> p r e", e=128)[:, :, HD:HD + 1]
                nc.vector.tensor_copy(
                    rs[:, :].rearrange("p (r o) -> p r o", o=1), sums
                )
                nc.vector.reciprocal_approx_fast(out=rc[:, :], in_=rs[:, :])
                return rc

            def drain_norm_half(w, oacc, rc, onorm, half, eng):
                """onorm-half = oacc regions (4 at once) * (1/rowsum): one
                broadcast tensor_mul per engine half (DVE: 0-3, Pool: 4-7) —
                a single instruction avoids the sync pass chaining 8 little
                muls across engines."""
                on = onorm[half]
                src = oacc[:, :].rearrange("p (r e) -> p r e", e=128)[
                    :, 4 * half:4 * half + 4, 0:HD
                ]
                scal = rc[:, 4 * half:4 * half + 4].unsqueeze(-1).broadcast_to(
                    [128, 4, HD]
                )
                eng.tensor_mul(
                    on[:, :].rearrange("p (r e) -> p r e", e=HD), src, scal
                )

            _tpw = {}

            def drain_tp(w, onorm, qsubs, finish):
                """transpose heads' [128q,64] blocks into a shared PSUM tile;
                one bulk copy into onm2 after the last pair."""
                hp, qw = w % 2, w // 2
                if w not in _tpw:
                    _tpw[w] = pop.tile([128, 512], BF, tag="po", name=f"tp_{w}")
                tp = _tpw[w]
                for qsub in qsubs:
                    for i in range(2):
                        r = 2 * qsub + i
                        on = onorm[r // 4]
                        nc.tensor.transpose(
                            tp[64 * i:64 * (i + 1), qsub * 128:(qsub + 1) * 128],
                            on[:, (r % 4) * HD:(r % 4 + 1) * HD],
                            ident[:, :],
                            tile_position=(0, 64 * i),
                        )
                if finish:
                    del _tpw[w]
                    nc.vector.tensor_copy(
                        onm2_sb[:, hp * S + qw * 512: hp * S + (qw + 1) * 512],
                        tp[:, :],
                    )

            # ---------------- lead-in ----------------
            # k-major accumulation pipelined against the serial xT DMA stream:
            # kt p0 nt0-3 (scp regions), qt p0 nt0, V s-chunks 0-5.
            ktl = [scp.tile([128, 1024], F32, tag="sc", name=f"lead_kt{i}")
                   for i in range(2)]
            qtl = pjp.tile([128, 512], F32, tag="pj", name="lead_qt")
            vl0 = opp.tile([128, 1024], F32, tag="oacc", name="lead_v01")
            vl1 = pop.tile([128, 512], F32, tag="po", name="lead_v2")

            def lead_kt_ap(nt):
                return ktl[nt // 2][:, (nt % 2) * 512:(nt % 2 + 1) * 512]

            # a matmul with start=True zeroes its whole 2KB bank on HW, so
            # zero the three lead V banks once up front and accumulate two
            # 256-col V regions per bank with start=False
            NVL = 6
            for bank, ap in enumerate(
                (vl0[:, 0:512], vl0[:, 512:1024], vl1[:, 0:512])
            ):
                nc.tensor.matmul(
                    ap, lhsT=zt[:, 0:128], rhs=zt[:, :],
                    start=True, stop=False, skip_group_check=True,
                )

            def lead_v_ap(c):
                if c < 4:
                    return vl0[:, c * 256:(c + 1) * 256]
                return vl1[:, (c - 4) * 256:(c - 3) * 256]

            def lead_kt_mm(k, nt):
                nc.tensor.matmul(
                    lead_kt_ap(nt),
                    lhsT=wk_sb[:, k * DHC: k * DHC + 128],
                    rhs=xT_sb[:, k * S + nt * 512: k * S + (nt + 1) * 512],
                    start=(k == 0),
                    stop=(k == KC - 1 and not with_bias),
                )

            for k in range(KC):
                for nt in range(4):
                    lead_kt_mm(k, nt)
                qk_mm(qtl, wq_sb, 0, 0, k)
                if k < KC - 2:
                    for c in range(NVL):
                        v_mm(None, c, k, ap=lead_v_ap(c), no_start=True)
            # the last two k-iterations' V matmuls are deferred (and
            # deprioritized) so the kt/qt fins -> first QK -> first exp chain
            # isn't stuck behind them in the static PE stream; they fill W0's
            # early PE slack instead.
            with tc.high_priority(offset=-70):
                for k in (KC - 2, KC - 1):
                    for c in range(NVL):
                        v_mm(None, c, k, ap=lead_v_ap(c), no_start=True)
            # fins spread across ACT/DVE/Pool so W0 can start ASAP; kt nt0/nt1
            # first (frees ktl[0] = the sc buffer W0 c0 needs).
            for nt in range(4):
                if with_bias:
                    nc.tensor.matmul(
                        lead_kt_ap(nt),
                        lhsT=bk[:, 0:128],
                        rhs=ones16[0:1, :],
                        start=False,
                        stop=True,
                    )
            if with_bias:
                nc.tensor.matmul(
                    qtl[:, :], lhsT=bq[:, 0:128], rhs=ones16[0:1, :],
                    start=False, stop=True,
                )
            nc.scalar.copy(qt_sb[:, 0:512], qtl[:, :])
            nc.scalar.copy(kt_sb[:, 0:512], lead_kt_ap(0))
            nc.scalar.copy(kt_sb[:, 512:1024], lead_kt_ap(1))
            nc.vector.tensor_copy(kt_sb[:, 1024:1536], lead_kt_ap(2))
            nc.vector.tensor_copy(kt_sb[:, 1536:2048], lead_kt_ap(3))
            for c in range(NVL):
                v_fin(lead_v_ap(c), c, eng=(nc.scalar if c % 2 else nc.vector))

            # ---------------- filler schedule ----------------
            # PE runs in program order: each (window, slot) gets at most
            # ~400ns of filler matmul work so a slot's PE time stays under
            # the 1038ns exp that paces the window (W0 excepted: it must
            # absorb the V chunks + W1's kt/qt and runs PE-bound).
            fillers = {}

            def add(w, c, fn):
                if c >= SB:
                    w, c = w + 1, c - SB
                fillers.setdefault((w, c), []).append(fn)

            def add_qk_spread(w, c0, per, dst, w_sb, bias, p, nt, fin_eng=None,
                             pool=None):
                """Project q/k unit (p, nt): KC chunk-matmuls at `per`/slot,
                fin copy the slot after the last chunk."""
                box = {}
                nsl = (KC + per - 1) // per
                for i in range(nsl):
                    ks = list(range(i * per, min(KC, (i + 1) * per)))
                    add(w, c0 + i,
                        lambda ks=ks: qk_chunk(box, w_sb, bias, p, nt, ks, pool))

                def fin():
                    ps = box.pop("ps")
                    qk_fin(ps, dst, bias, p, nt, on_act=(fin_eng == "act"))
                add(w, c0 + nsl, fin)

            def add_po_spread(w, c0, sb, n):
                """Outproj piece: 1 matmul/slot + fin."""
                box = {}

                def m(p):
                    with tc.high_priority(offset=-90):
                        if p == 0:
                            box["po"] = pop.tile(
                                [128, 512], F32, tag="po", name=f"po_{sb}_{n}"
                            )
                        outproj_mm(box["po"], sb, n, p)
                add(w, c0, lambda: m(0))
                add(w, c0 + 1, lambda: m(1))
                add(w, c0 + 1, lambda: outproj_fin(box.pop("po"), sb, n))

            # W0: V s-chunks 6..15 (one per slot, JIT for its own PV), then
            # kt/qt p1 units W1 needs (PE-bound window; chunky is fine).
            for c in range(NVL, SB):
                add(0, c - 4, lambda c=c: proj_v(c, pool=(pop if c % 2 else pjp)))
            add_qk_spread(0, 12, 4, kt_sb, wk_sb, bk, 1, 0)
            add_qk_spread(0, 13, 4, qt_sb, wq_sb, bq, 1, 0, pool=pop)
            # W1: kt p1 nt1-nt3 (own use), qt p0 nt1 (W2), qt p1 nt1 (W3)
            add_qk_spread(1, 0, 4, kt_sb, wk_sb, bk, 1, 1)
            add_qk_spread(1, 2, 2, kt_sb, wk_sb, bk, 1, 2, pool=pop)
            add_qk_spread(1, 6, 2, kt_sb, wk_sb, bk, 1, 3)
            add_qk_spread(1, 8, 2, qt_sb, wq_sb, bq, 0, 1, pool=pop)
            add_qk_spread(1, 12, 2, qt_sb, wq_sb, bq, 1, 1)
            # later q-window projections, two windows ahead of use
            add_qk_spread(2, 4, 2, qt_sb, wq_sb, bq, 0, 2)
            add_qk_spread(3, 4, 2, qt_sb, wq_sb, bq, 1, 2)
            add_qk_spread(4, 4, 2, qt_sb, wq_sb, bq, 0, 3)
            add_qk_spread(5, 4, 2, qt_sb, wq_sb, bq, 1, 3)
            # outproj for q-block qw at windows 2qw+2 / 2qw+3 (qw 0..2)
            po_slots = (3, 6, 9, 12)
            for qw in range(3):
                for i in range(8):
                    sb, n = 4 * qw + i // 2, i % 2
                    w = 2 * qw + 2 + i // 4
                    add_po_spread(w, po_slots[i % 4], sb, n)

            # ---------------- attention windows ----------------
            pending = []  # windows whose oacc awaits drain

            def drain_step(w, oacc, onorm, step):
                # all 8 norm-muls at step 0 (DVE/Pool split) so the oacc
                # buffer frees before the next window's first PV write.
                if step == 0:
                    rc = drain_sums(w, oacc)
                    drain_norm_half(w, oacc, rc, onorm, 0, nc.vector)
                    drain_norm_half(w, oacc, rc, onorm, 1, nc.vector)
                elif step == 1:
                    drain_tp(w, onorm, (0, 1), False)
                elif step == 2:
                    drain_tp(w, onorm, (2, 3), True)

            carry = []  # previous window's (emit_pv, pt) for its last chunk:
            # emitted after the NEXT window's first QK so the in-order PE
            # stream doesn't serialize QK(c0') behind exp(c15).

            for w in range(NW):
                qw, hp = w // 2, w % 2
                oacc = opp.tile([128, 1024], F32, tag="oacc", name=f"o_{w}")
                onorm = (
                    onp.tile([128, 256], BF, tag="onA", name=f"onA_{w}"),
                    onp.tile([128, 256], BF, tag="onB", name=f"onB_{w}"),
                )
                prev = None

                for bank in range(2):
                    # start=True zeroes the full 2KB bank on HW; the 4 oacc
                    # accumulation regions per bank then run start=False
                    nc.tensor.matmul(
                        oacc[:, bank * 512:(bank + 1) * 512],
                        lhsT=zt[:, 0:128],
                        rhs=zt[:, :],
                        start=True,
                        stop=False,
                        skip_group_check=True,
                    )

                def emit_pv(pt_t, c, oacc=oacc, hp=hp):
                    # deprioritized: PV fills PE idle; ready QK/exp (the
                    # window pacer) must never queue behind a PV burst
                    with tc.high_priority(offset=-60):
                        for qsub in range(4):
                            for i in range(2):
                                r = 2 * qsub + i
                                nc.tensor.matmul(
                                    oacc[:, r * 128: r * 128 + HD + 1],
                                    lhsT=pt_t[:, i * 512 + qsub * 128:
                                              i * 512 + (qsub + 1) * 128],
                                    rhs=v_sb[:, c * VP + (2 * hp + i) * (HD + 1):
                                             c * VP + (2 * hp + i + 1) * (HD + 1)],
                                    start=False,
                                    stop=(c == SB - 1),
                                    skip_group_check=True,
                                )

                for c in range(SB):
                    sc = scp.tile([128, 1024], F32, tag="sc", name=f"sc_{w}_{c}")
                    for i in range(2):  # head A | head B packed
                        nc.tensor.matmul(
                            sc[:, 512 * i:512 * (i + 1)],
                            lhsT=kt_sb[
                                64 * i:64 * (i + 1),
                                hp * S + c * 128: hp * S + (c + 1) * 128,
                            ],
                            rhs=qt_sb[
                                64 * i:64 * (i + 1),
                                hp * S + qw * 512: hp * S + (qw + 1) * 512,
                            ],
                            start=True,
                            stop=True,
                        )
                    pt_t = ptp.tile([128, 1024], BF, tag="pt", name=f"pt_{w}_{c}")
                    nc.scalar.activation(pt_t[:, :], sc[:, :], EXP, scale=0.125)
                    if c == 0 and carry:
                        fn, pt_prev = carry.pop()
                        fn(pt_prev, SB - 1)
                    if pending and c <= 2:
                        with tc.high_priority(offset=-60):
                            drain_step(*pending[0], c)
                        if c == 2:
                            pending.pop(0)
                    for fn in fillers.get((w, c), ()):
                        fn()
                    if prev is not None:
                        emit_pv(prev, c - 1)
                    prev = pt_t
                carry.append((emit_pv, prev))
                pending.append((w, oacc, onorm))

            # tail: drain last window + outproj for q-block 3. The attention
            # PSUM pools are idle now — draw the po tiles from scp/opp too so
            # the 8 pieces pipeline without ring-reuse stalls.
            fn, pt_prev = carry.pop()
            fn(pt_prev, SB - 1)
            w, oacc, onorm = pending.pop(0)
            hp, qw = w % 2, w // 2
            rc = drain_sums(w, oacc)
            # both norm halves on DVE: engine-internal ordering is free; a
            # DVE->Pool handoff would sit on the tail critical path
            drain_norm_half(w, oacc, rc, onorm, 0, nc.vector)
            drain_norm_half(w, oacc, rc, onorm, 1, nc.vector)
            tail_sc = [scp.tile([128, 1024], F32, tag="sc", name=f"tsc{i}")
                       for i in range(2)]
            tail_o = opp.tile([128, 1024], F32, tag="oacc", name="tail_o")
            tail_po = [
                tail_sc[0][:, 0:512], tail_o[:, 0:512],
                tail_sc[0][:, 512:1024], tail_o[:, 512:1024],
                tail_sc[1][:, 0:512], tail_sc[0][:, 0:512],
                tail_sc[1][:, 512:1024], tail_sc[0][:, 512:1024],
            ]
            tpt = pop.tile([128, 512], BF, tag="po", name="tp_tail")
            for qsub in range(4):
                for i in range(2):
                    r = 2 * qsub + i
                    on = onorm[r // 4]
                    nc.tensor.transpose(
                        tpt[64 * i:64 * (i + 1), qsub * 128:(qsub + 1) * 128],
                        on[:, (r % 4) * HD:(r % 4 + 1) * HD],
                        ident[:, :],
                        tile_position=(0, 64 * i),
                    )
                col = hp * S + qw * 512 + qsub * 128
                if qsub % 2:
                    nc.vector.tensor_copy(
                        onm2_sb[:, col:col + 128],
                        tpt[:, qsub * 128:(qsub + 1) * 128],
                    )
                else:
                    nc.scalar.copy(
                        onm2_sb[:, col:col + 128],
                        tpt[:, qsub * 128:(qsub + 1) * 128],
                    )
            # all onm2 blocks staged; now the 8 pieces pipeline at copy rate
            for qsub in range(4):
                for n in range(2):
                    outproj_piece(
                        12 + qsub, n, on_act=(n == 0),
                        po=tail_po[2 * qsub + n],
                    )

            if debug:
                nc.sync.dma_start(dbg["qt"][:, :], qt_sb[:, :])
                nc.sync.dma_start(dbg["kt"][:, :], kt_sb[:, :])
                nc.sync.dma_start(dbg["v"][:, :], v_sb[:, :])
                nc.sync.dma_start(dbg["onm2"][:, :], onm2_sb[:, :])

    nc.compile()
    return nc


def _get_nc(with_bias=False):
    if with_bias not in _nc_cache:
        _nc_cache[with_bias] = _build_bass(with_bias=with_bias)
    return _nc_cache[with_bias]


def _prepare_in_maps(x, wq, bq, wk, bk, wv, bv, wo):
    import ml_dtypes

    bf16 = ml_dtypes.bfloat16
    x = np.asarray(x, np.float32)
    wq, bq = np.asarray(wq, np.float32), np.asarray(bq, np.float32)
    wk, bk = np.asarray(wk, np.float32), np.asarray(bk, np.float32)
    wv, bv = np.asarray(wv, np.float32), np.asarray(bv, np.float32)
    wo = np.asarray(wo, np.float32)

    xT = [np.ascontiguousarray(x[b].T).astype(bf16) for b in range(B)]
    in_maps = []
    for c in range(NC):
        b, j = divmod(c, HPC)
        cs = slice(DHC * j, DHC * (j + 1))
        bias3 = np.concatenate([bq[cs], bk[cs], bv[cs]]).reshape(1, 3 * DHC).astype(bf16)
        in_maps.append(
            {
                "xT": xT[b],
                "wq_c": np.ascontiguousarray(wq[:, cs]).astype(bf16),
                "wk_c": np.ascontiguousarray(wk[:, cs]).astype(bf16),
                "wv_c": np.ascontiguousarray(wv[:, cs]).astype(bf16),
                "wo_c": np.ascontiguousarray(wo[cs, :]).astype(bf16),
                "bias3": np.ascontiguousarray(bias3),
            }
        )
    return in_maps


def _gather(parts, bo):
    bo = np.asarray(bo, np.float32)
    out = np.empty((B, S, D), np.float32)
    for b in range(B):
        acc = np.asarray(parts[HPC * b], np.float32)
        for j in range(1, HPC):
            acc = acc + np.asarray(parts[HPC * b + j], np.float32)
        out[b] = acc + bo
    return out


def kernel(x, wq, bq, wk, bk, wv, bv, wo, bo):
    from concourse import bass_utils

    in_maps = _prepare_in_maps(x, wq, bq, wk, bk, wv, bv, wo)
    with_bias = bool(
        np.any(np.asarray(bq)) or np.any(np.asarray(bk)) or np.any(np.asarray(bv))
    )
    res = bass_utils.run_bass_kernel_spmd(
        nc=_get_nc(with_bias), in_maps=in_maps, core_ids=list(range(NC))
    )
    parts = [np.asarray(r["out"], np.float32) for r in res.results]
    return _gather(parts, bo)


# revision 79
# speedup vs baseline: 1.0000x; 1.0000x over previous
"""Trainium2 Bass kernel for nn_Attention_80384607912675.

Multi-head attention (B=2, S=2048, D=1024, H=16, HD=64), fp32 reference.

Sharding (8 cores): data-parallel over batch (2) x tensor-parallel over heads
(4 head groups of 4 heads).  Core c handles batch c//4, heads [4*(c%4), 4*(c%4)+4).
wq/wk/wv split column-wise, wo split row-wise; the wo partial sums are reduced
on the host.

Per-core kernel (all matmuls bf16 with fp32 PSUM accumulation):
  QT/KT = (x @ wq/k)^T  stored head-major [256 -> 2x128, 2048]
  V_aug = [x @ wv | 1]  stored NATURAL [2048, 4*(64+1)] (no transposes: the
                        projection emits [s,d] tiles directly; ones column per
                        head folds the softmax row-sum into PV)
  per window w = (qw, hp) (q-window 512 wide, head pair hp):
    S^T[kp, q] = K_h^T (x) Q_h  (A,B packed in one [128,1024] PSUM tile)
    P^T        = exp(S^T / 8)   (one ScalarE instr per A|B pair, ->bf16)
    O[q, d]   += P^T_slice^T (x) V_aug  (O NATURAL: out [128q, 65] regions,
                128-partition output, ~2x cheaper on PE than the O^T form)
  normalization is per-PARTITION (rowsum col 64 of each region): one DVE
  tensor_scalar (mult by reciprocal AP) per region -> bf16, then PE-transposed
  into O^T (onm2) for the output projection (head B lands at PSUM base 64 via
  tile_position so no partition-relocation DMA is needed).
  outproj: ONE fused [2048,1024] partial per core (both pair blocks
  accumulated in PSUM), DMA'd bf16; host adds 4 partials per batch + bo.

PE is the bottleneck (~140us busy); everything else (exp stream on ACT ~133us,
copies/norm on DVE, DMA) hides under it except the serial input-DMA lead-in.
"""

import numpy as np

B, S, D, H = 2, 2048, 1024, 16
HD = D // H          # 64
HPC = 4              # heads per core
DHC = HPC * HD       # 256 head dims per core
KC = D // 128        # 8 contraction chunks
SB = S // 128        # 16 s blocks / kp chunks
VP = HPC * (HD + 1)  # 260: V storage pitch per s-chunk (ones col per head)
NC = 8               # cores
NQW = 4              # 512-wide q windows per head pair
NW = 8               # windows: w -> (qw = w//2, hp = w%2)

_nc_cache = {}


def _build_bass(with_bias=False, debug=False):
    import concourse.mybir as mybir
    import concourse.tile as tile
    from concourse import bacc

    BF = mybir.dt.bfloat16
    F32 = mybir.dt.float32
    EXP = mybir.ActivationFunctionType.Exp

    nc = bacc.Bacc("TRN2")

    xT_d = nc.dram_tensor("xT", [D, S], BF, kind="ExternalInput")
    wq_d = nc.dram_tensor("wq_c", [D, DHC], BF, kind="ExternalInput")
    wk_d = nc.dram_tensor("wk_c", [D, DHC], BF, kind="ExternalInput")
    wv_d = nc.dram_tensor("wv_c", [D, DHC], BF, kind="ExternalInput")
    wo_d = nc.dram_tensor("wo_c", [DHC, D], BF, kind="ExternalInput")
    bias_d = nc.dram_tensor("bias3", [1, 3 * DHC], BF, kind="ExternalInput")
    out_d = nc.dram_tensor("out", [S, D], BF, kind="ExternalOutput")
    if debug:
        dbg = {
            "qt": nc.dram_tensor("dbg_qt", [128, 2 * S], BF, kind="ExternalOutput"),
            "kt": nc.dram_tensor("dbg_kt", [128, 2 * S], BF, kind="ExternalOutput"),
            "v": nc.dram_tensor("dbg_v", [128, SB * VP], BF, kind="ExternalOutput"),
            "onm2": nc.dram_tensor(
                "dbg_onm2", [128, 2 * S], BF, kind="ExternalOutput"
            ),
        }

    with tile.TileContext(nc) as tc:
        with (
            tc.tile_pool(name="persist", bufs=1) as pp,
            tc.tile_pool(name="sc", bufs=2, space="PSUM") as scp,
            tc.tile_pool(name="oacc", bufs=1, space="PSUM") as opp,
            tc.tile_pool(name="pj", bufs=1, space="PSUM") as pjp,
            tc.tile_pool(name="po", bufs=1, space="PSUM") as pop,
            tc.tile_pool(name="pt", bufs=16) as ptp,
            tc.tile_pool(name="onorm", bufs=3) as onp,
            tc.tile_pool(name="rc", bufs=4) as rcp,
            tc.tile_pool(name="osb", bufs=8) as oup,
        ):
            # PSUM budget (16KB/partition = 8 banks): sc 2x[128,1024]f32 +
            # oacc [128,1024]f32 + pj [128,512]f32 (projection accum) +
            # po [128,512]f32 (outproj accum / transpose scratch). The PE
            # runs in program order, so filler work is dripped in <=2-matmul
            # bites per slot to keep each slot's PE time under the 1038ns
            # ACT exp that paces the steady-state window.
            xT_sb = pp.tile([128, KC * S], BF, tag="xT", name="xT_sb")
            wq_sb = pp.tile([128, KC * DHC], BF, tag="wq", name="wq_sb")
            wk_sb = pp.tile([128, KC * DHC], BF, tag="wk", name="wk_sb")
            wv_sb = pp.tile([128, KC * DHC], BF, tag="wv", name="wv_sb")
            wo_sb = pp.tile([128, 2 * D], BF, tag="wo", name="wo_sb")
            qt_sb = pp.tile([128, 2 * S], BF, tag="qt", name="qt_sb")
            kt_sb = pp.tile([128, 2 * S], BF, tag="kt", name="kt_sb")
            v_sb = pp.tile([128, SB * VP], BF, tag="v", name="v_sb")
            onm2_sb = pp.tile([128, 2 * S], BF, tag="onm2", name="onm2_sb")
            ident = pp.tile([128, 128], BF, tag="ident", name="ident")
            bias_sb = pp.tile([1, 3 * DHC], BF, tag="bias", name="bias_sb")
            ones16 = pp.tile([1, 512], BF, tag="ones16", name="ones16")

            # input DMAs (DMA is serial: order = arrival order). Weights
            # needed by the k-major lead-in come first, then the xT chunks
            # consumed per k; wo last (needed late).
            def load_w(w_sb, w_d):
                nc.sync.dma_start(
                    w_sb[:, :].rearrange("p (k d) -> p k d", d=DHC),
                    w_d[:, :].rearrange("(k p) d -> p k d", p=128),
                )

            def load_xt(k):
                nc.sync.dma_start(
                    xT_sb[:, k * S:(k + 1) * S], xT_d[k * 128:(k + 1) * 128, :]
                )

            def load_xt_half(k, h):
                nc.sync.dma_start(
                    xT_sb[:, k * S + h * 1024:(k * S) + (h + 1) * 1024],
                    xT_d[k * 128:(k + 1) * 128, h * 1024:(h + 1) * 1024],
                )

            load_w(wk_sb, wk_d)
            load_w(wq_sb, wq_d)
            load_xt(0)
            load_w(wv_sb, wv_d)
            for k in range(1, KC - 1):
                load_xt(k)
            # split the last chunk so the lead's final k-iteration (nt0/nt1/
            # qt/V need only cols 0-1023) starts a half-chunk earlier
            load_xt_half(KC - 1, 0)
            load_xt_half(KC - 1, 1)
            nc.sync.dma_start(bias_sb[:, :], bias_d[:, :])
            nc.sync.dma_start(
                wo_sb[:, :].rearrange("r (p d) -> r p d", d=D),
                wo_d[:, :].rearrange("(p r) d -> r p d", r=128),
            )
            zt = pp.tile([128, 512], BF, tag="zt", name="zt")
            nc.vector.memset(ones16[:, :], 1.0)
            nc.vector.memset(zt[:, :], 0.0)
            # ones columns of V_aug: preset everything to 1, V overwrites below
            nc.gpsimd.memset(v_sb[:, :], 1.0)
            from concourse.masks import make_identity
            make_identity(nc, ident[:, :])

            bq = bias_sb[0:1, 0:DHC]
            bk = bias_sb[0:1, DHC:2 * DHC]
            bv = bias_sb[0:1, 2 * DHC:3 * DHC]

            # ---------------- projection helpers ----------------
            def qk_mm(ps, w_sb, p, nt, k):
                nc.tensor.matmul(
                    ps[:, :],
                    lhsT=w_sb[:, k * DHC + p * 128: k * DHC + (p + 1) * 128],
                    rhs=xT_sb[:, k * S + nt * 512: k * S + (nt + 1) * 512],
                    start=(k == 0),
                    stop=(k == KC - 1 and not with_bias),
                )

            def qk_fin(ps, dst, bias, p, nt, on_act=False):
                if with_bias:
                    nc.tensor.matmul(
                        ps[:, :],
                        lhsT=bias[:, p * 128:(p + 1) * 128],
                        rhs=ones16[0:1, :],
                        start=False,
                        stop=True,
                    )
                dslice = dst[:, p * S + nt * 512: p * S + (nt + 1) * 512]
                if on_act:
                    nc.scalar.copy(dslice, ps[:, :])
                else:
                    nc.vector.tensor_copy(dslice, ps[:, :])

            def qk_chunk(box, w_sb, bias, p, nt, ks, pool=None):
                if 0 in ks:
                    pool = pool or pjp
                    box["ps"] = pool.tile(
                        [128, 512], F32, tag=pool.name, name=f"qk_{p}_{nt}"
                    )
                for k in ks:
                    qk_mm(box["ps"], w_sb, p, nt, k)

            def v_mm(ps, c, k, ap=None, no_start=False):
                nc.tensor.matmul(
                    ps if ap is None else ap,
                    lhsT=xT_sb[:, k * S + c * 128: k * S + (c + 1) * 128],
                    rhs=wv_sb[:, k * DHC:(k + 1) * DHC],
                    start=(k == 0 and not no_start),
                    stop=(k == KC - 1 and not with_bias),
                    skip_group_check=no_start,
                )

            def v_fin(ps_ap, c, eng=None):
                if with_bias:
                    nc.tensor.matmul(
                        ps_ap,
                        lhsT=ones16[0:1, 0:128],
                        rhs=bv[:, :],
                        start=False,
                        stop=True,
                    )
                dst3 = v_sb[
                    :, c * VP:(c + 1) * VP
                ].rearrange("p (h e) -> p h e", e=HD + 1)[:, :, 0:HD]
                if eng is nc.scalar:
                    nc.scalar.copy(dst3, ps_ap)
                else:
                    (eng or nc.vector).tensor_copy(dst3, ps_ap)

            def proj_v(c, eng=None, pool=None):
                """V s-chunk c: [128 s, 256 d] natural, all K chunks, + fin."""
                pool = pool or pjp
                ps = pool.tile([128, 512], F32, tag=pool.name, name=f"v_{c}")
                ap = ps[:, 0:DHC]
                for k in range(KC):
                    v_mm(None, c, k, ap=ap)
                v_fin(ap, c, eng=eng)

            # ---------------- outproj ----------------
            def outproj_mm(po, sb, n, p):
                nc.tensor.matmul(
                    po[:, :],
                    lhsT=onm2_sb[:, p * S + sb * 128: p * S + (sb + 1) * 128],
                    rhs=wo_sb[:, p * D + n * 512: p * D + (n + 1) * 512],
                    start=(p == 0),
                    stop=(p == 1),
                )

            _ot_cache = {}

            def outproj_fin(po, sb, n, on_act=False, eng=None):
                # both n-halves of an sb share one SBUF tile and one DMA:
                # halves the HWDGE count so the tail's DMA stream drains fast
                if sb not in _ot_cache:
                    _ot_cache[sb] = oup.tile(
                        [128, 1024], BF, tag="osb", name=f"ot_{sb}"
                    )
                ot = _ot_cache[sb]
                if on_act:
                    nc.scalar.copy(ot[:, n * 512:(n + 1) * 512], po[:, :])
                else:
                    (eng or nc.vector).tensor_copy(
                        ot[:, n * 512:(n + 1) * 512], po[:, :]
                    )
                if sb in _ot_done:
                    del _ot_cache[sb]
                    nc.sync.dma_start(
                        out_d[sb * 128:(sb + 1) * 128, :], ot[:, :]
                    )
                else:
                    _ot_done.add(sb)

            _ot_done = set()

            def outproj_piece(sb, n, on_act=False, po=None, eng=None):
                if po is None:
                    po = pop.tile([128, 512], F32, tag="po", name=f"po_{sb}_{n}")
                for p in range(2):
                    outproj_mm(po, sb, n, p)
                outproj_fin(po, sb, n, on_act, eng)

            # ---------------- drain (normalize + transpose) ----------------
            def drain_sums(w, oacc):
                """rowsum cols -> reciprocal (one DVE op for all 8 regions)."""
                rs = rcp.tile([128, 8], F32, tag="rc", name=f"rs_{w}")
                rc = rcp.tile([128, 8], F32, tag="rc", name=f"rc_{w}")
                sums = oacc[:, :].rearrange("p (r e) -> p r e", e=128)[:, :, HD:HD + 1]
                nc.vector.tensor_copy(
                    rs[:, :].rearrange("p (r o) -> p r o", o=1), sums
                )
                nc.vector.reciprocal_approx_fast(out=rc[:, :], in_=rs[:, :])
                return rc

            def drain_norm_half(w, oacc, rc, onorm, half, eng):
                """onorm-half = oacc regions (4 at once) * (1/rowsum): one
                broadcast tensor_mul per engine half (DVE: 0-3, Pool: 4-7) —
                a single instruction avoids the sync pass chaining 8 little
                muls across engines."""
                on = onorm[half]
                src = oacc[:, :].rearrange("p (r e) -> p r e", e=128)[
                    :, 4 * half:4 * half + 4, 0:HD
                ]
                scal = rc[:, 4 * half:4 * half + 4].unsqueeze(-1).broadcast_to(
                    [128, 4, HD]
                )
                eng.tensor_mul(
                    on[:, :].rearrange("p (r e) -> p r e", e=HD), src, scal
                )

            _tpw = {}

            def drain_tp(w, onorm, qsubs, finish):
                """transpose heads' [128q,64] blocks into a shared PSUM tile;
                one bulk copy into onm2 after the last pair."""
                hp, qw = w % 2, w // 2
                if w not in _tpw:
                    _tpw[w] = pop.tile([128, 512], BF, tag="po", name=f"tp_{w}")
                tp = _tpw[w]
                for qsub in qsubs:
                    for i in range(2):
                        r = 2 * qsub + i
                        on = onorm[r // 4]
                        nc.tensor.transpose(
                            tp[64 * i:64 * (i + 1), qsub * 128:(qsub + 1) * 128],
                            on[:, (r % 4) * HD:(r % 4 + 1) * HD],
                            ident[:, :],
                            tile_position=(0, 64 * i),
                        )
                if finish:
                    del _tpw[w]
                    nc.vector.tensor_copy(
                        onm2_sb[:, hp * S + qw * 512: hp * S + (qw + 1) * 512],
                        tp[:, :],
                    )

            # ---------------- lead-in ----------------
            # k-major accumulation pipelined against the serial xT DMA stream:
            # kt p0 nt0-3 (scp regions), qt p0 nt0, V s-chunks 0-5.
            ktl = [scp.tile([128, 1024], F32, tag="sc", name=f"lead_kt{i}")
                   for i in range(2)]
            qtl = pjp.tile([128, 512], F32, tag="pj", name="lead_qt")
            vl0 = opp.tile([128, 1024], F32, tag="oacc", name="lead_v01")
            vl1 = pop.tile([128, 512], F32, tag="po", name="lead_v2")

            def lead_kt_ap(nt):
                return ktl[nt // 2][:, (nt % 2) * 512:(nt % 2 + 1) * 512]

            # a matmul with start=True zeroes its whole 2KB bank on HW, so
            # zero the three lead V banks once up front and accumulate two
            # 256-col V regions per bank with start=False
            NVL = 6
            for bank, ap in enumerate(
                (vl0[:, 0:512], vl0[:, 512:1024], vl1[:, 0:512])
            ):
                nc.tensor.matmul(
                    ap, lhsT=zt[:, 0:128], rhs=zt[:, :],
                    start=True, stop=False, skip_group_check=True,
                )

            def lead_v_ap(c):
                if c < 4:
                    return vl0[:, c * 256:(c + 1) * 256]
                return vl1[:, (c - 4) * 256:(c - 3) * 256]

            def lead_kt_mm(k, nt):
                nc.tensor.matmul(
                    lead_kt_ap(nt),
                    lhsT=wk_sb[:, k * DHC: k * DHC + 128],
                    rhs=xT_sb[:, k * S + nt * 512: k * S + (nt + 1) * 512],
                    start=(k == 0),
                    stop=(k == KC - 1 and not with_bias),
                )

            for k in range(KC):
                for nt in range(4):
                    lead_kt_mm(k, nt)
                qk_mm(qtl, wq_sb, 0, 0, k)
                if k < KC - 2:
                    for c in range(NVL):
                        v_mm(None, c, k, ap=lead_v_ap(c), no_start=True)
            # the last two k-iterations' V matmuls are deferred (and
            # deprioritized) so the kt/qt fins -> first QK -> first exp chain
            # isn't stuck behind them in the static PE stream; they fill W0's
            # early PE slack instead.
            with tc.high_priority(offset=-70):
                for k in (KC - 2, KC - 1):
                    for c in range(NVL):
                        v_mm(None, c, k, ap=lead_v_ap(c), no_start=True)
            # fins spread across ACT/DVE/Pool so W0 can start ASAP; kt nt0/nt1
            # first (frees ktl[0] = the sc buffer W0 c0 needs).
            for nt in range(4):
                if with_bias:
                    nc.tensor.matmul(
                        lead_kt_ap(nt),
                        lhsT=bk[:, 0:128],
                        rhs=ones16[0:1, :],
                        start=False,
                        stop=True,
                    )
            if with_bias:
                nc.tensor.matmul(
                    qtl[:, :], lhsT=bq[:, 0:128], rhs=ones16[0:1, :],
                    start=False, stop=True,
                )
            nc.scalar.copy(qt_sb[:, 0:512], qtl[:, :])
            nc.scalar.copy(kt_sb[:, 0:512], lead_kt_ap(0))
            nc.scalar.copy(kt_sb[:, 512:1024], lead_kt_ap(1))
            nc.vector.tensor_copy(kt_sb[:, 1024:1536], lead_kt_ap(2))
            nc.vector.tensor_copy(kt_sb[:, 1536:2048], lead_kt_ap(3))
            for c in range(NVL):
                v_fin(lead_v_ap(c), c, eng=(nc.scalar if c % 2 else nc.vector))

            # ---------------- filler schedule ----------------
            # PE runs in program order: each (window, slot) gets at most
            # ~400ns of filler matmul work so a slot's PE time stays under
            # the 1038ns exp that paces the window (W0 excepted: it must
            # absorb the V chunks + W1's kt/qt and runs PE-bound).
            fillers = {}

            def add(w, c, fn):
                if c >= SB:
                    w, c = w + 1, c - SB
                fillers.setdefault((w, c), []).append(fn)

            def add_qk_spread(w, c0, per, dst, w_sb, bias, p, nt, fin_eng=None,
                             pool=None):
                """Project q/k unit (p, nt): KC chunk-matmuls at `per`/slot,
                fin copy the slot after the last chunk."""
                box = {}
                nsl = (KC + per - 1) // per
                for i in range(nsl):
                    ks = list(range(i * per, min(KC, (i + 1) * per)))
                    add(w, c0 + i,
                        lambda ks=ks: qk_chunk(box, w_sb, bias, p, nt, ks, pool))

                def fin():
                    ps = box.pop("ps")
                    qk_fin(ps, dst, bias, p, nt, on_act=(fin_eng == "act"))
                add(w, c0 + nsl, fin)

            def add_po_spread(w, c0, sb, n):
                """Outproj piece: 1 matmul/slot + fin."""
                box = {}

                def m(p):
                    with tc.high_priority(offset=-90):
                        if p == 0:
                            box["po"] = pop.tile(
                                [128, 512], F32, tag="po", name=f"po_{sb}_{n}"
                            )
                        outproj_mm(box["po"], sb, n, p)
                add(w, c0, lambda: m(0))
                add(w, c0 + 1, lambda: m(1))
                add(w, c0 + 1, lambda: outproj_fin(box.pop("po"), sb, n))

            # W0: V s-chunks 6..15 (one per slot, JIT for its own PV), then
            # kt/qt p1 units W1 needs (PE-bound window; chunky is fine).
            for c in range(NVL, SB):
                add(0, c - 4, lambda c=c: proj_v(c, pool=(pop if c % 2 else pjp)))
            add_qk_spread(0, 12, 4, kt_sb, wk_sb, bk, 1, 0)
            add_qk_spread(0, 13, 4, qt_sb, wq_sb, bq, 1, 0, pool=pop)
            # W1: kt p1 nt1-nt3 (own use), qt p0 nt1 (W2), qt p1 nt1 (W3)
            add_qk_spread(1, 0, 4, kt_sb, wk_sb, bk, 1, 1)
            add_qk_spread(1, 2, 2, kt_sb, wk_sb, bk, 1, 2, pool=pop)
            add_qk_spread(1, 6, 2, kt_sb, wk_sb, bk, 1, 3)
            add_qk_spread(1, 8, 2, qt_sb, wq_sb, bq, 0, 1, pool=pop)
            add_qk_spread(1, 12, 2, qt_sb, wq_sb, bq, 1, 1)
            # later q-window projections, two windows ahead of use
            add_qk_spread(2, 4, 1, qt_sb, wq_sb, bq, 0, 2)
            add_qk_spread(3, 4, 1, qt_sb, wq_sb, bq, 1, 2)
            add_qk_spread(4, 4, 1, qt_sb, wq_sb, bq, 0, 3)
            add_qk_spread(5, 4, 1, qt_sb, wq_sb, bq, 1, 3)
            # outproj for q-block qw at windows 2qw+2 / 2qw+3 (qw 0..2)
            po_slots = (3, 6, 9, 12)
            for qw in range(3):
                for i in range(8):
                    sb, n = 4 * qw + i // 2, i % 2
                    w = 2 * qw + 2 + i // 4
                    add_po_spread(w, po_slots[i % 4], sb, n)

            # ---------------- attention windows ----------------
            pending = []  # windows whose oacc awaits drain

            def drain_step(w, oacc, onorm, step):
                # all 8 norm-muls at step 0 (DVE/Pool split) so the oacc
                # buffer frees before the next window's first PV write.
                if step == 0:
                    rc = drain_sums(w, oacc)
                    drain_norm_half(w, oacc, rc, onorm, 0, nc.vector)
                    drain_norm_half(w, oacc, rc, onorm, 1, nc.vector)
                elif step == 1:
                    drain_tp(w, onorm, (0, 1), False)
                elif step == 2:
                    drain_tp(w, onorm, (2, 3), True)

            carry = []  # previous window's (emit_pv, pt) for its last chunk:
            # emitted after the NEXT window's first QK so the in-order PE
            # stream doesn't serialize QK(c0') behind exp(c15).

            for w in range(NW):
                qw, hp = w // 2, w % 2
                oacc = opp.tile([128, 1024], F32, tag="oacc", name=f"o_{w}")
                onorm = (
                    onp.tile([128, 256], BF, tag="onA", name=f"onA_{w}"),
                    onp.tile([128, 256], BF, tag="onB", name=f"onB_{w}"),
                )
                prev = None

                for bank in range(2):
                    # start=True zeroes the full 2KB bank on HW; the 4 oacc
                    # accumulation regions per bank then run start=False
                    nc.tensor.matmul(
                        oacc[:, bank * 512:(bank + 1) * 512],
                        lhsT=zt[:, 0:128],
                        rhs=zt[:, :],
                        start=True,
                        stop=False,
                        skip_group_check=True,
                    )

                def emit_pv(pt_t, c, oacc=oacc, hp=hp):
                    # deprioritized: PV fills PE idle; ready QK/exp (the
                    # window pacer) must never queue behind a PV burst
                    with tc.high_priority(offset=-60):
                        for qsub in range(4):
                            for i in range(2):
                                r = 2 * qsub + i
                                nc.tensor.matmul(
                                    oacc[:, r * 128: r * 128 + HD + 1],
                                    lhsT=pt_t[:, i * 512 + qsub * 128:
                                              i * 512 + (qsub + 1) * 128],
                                    rhs=v_sb[:, c * VP + (2 * hp + i) * (HD + 1):
                                             c * VP + (2 * hp + i + 1) * (HD + 1)],
                                    start=False,
                                    stop=(c == SB - 1),
                                    skip_group_check=True,
                                )

                for c in range(SB):
                    sc = scp.tile([128, 1024], F32, tag="sc", name=f"sc_{w}_{c}")
                    for i in range(2):  # head A | head B packed
                        nc.tensor.matmul(
                            sc[:, 512 * i:512 * (i + 1)],
                            lhsT=kt_sb[
                                64 * i:64 * (i + 1),
                                hp * S + c * 128: hp * S + (c + 1) * 128,
                            ],
                            rhs=qt_sb[
                                64 * i:64 * (i + 1),
                                hp * S + qw * 512: hp * S + (qw + 1) * 512,
                            ],
                            start=True,
                            stop=True,
                        )
                    pt_t = ptp.tile([128, 1024], BF, tag="pt", name=f"pt_{w}_{c}")
                    nc.scalar.activation(pt_t[:, :], sc[:, :], EXP, scale=0.125)
                    if c == 0 and carry:
                        fn, pt_prev = carry.pop()
                        fn(pt_prev, SB - 1)
                    if pending and c <= 2:
                        with tc.high_priority(offset=-60):
                            drain_step(*pending[0], c)
                        if c == 2:
                            pending.pop(0)
                    for fn in fillers.get((w, c), ()):
                        fn()
                    if prev is not None:
                        emit_pv(prev, c - 1)
                    prev = pt_t
                carry.append((emit_pv, prev))
                pending.append((w, oacc, onorm))

            # tail: drain last window + outproj for q-block 3. The attention
            # PSUM pools are idle now — draw the po tiles from scp/opp too so
            # the 8 pieces pipeline without ring-reuse stalls.
            fn, pt_prev = carry.pop()
            fn(pt_prev, SB - 1)
            w, oacc, onorm = pending.pop(0)
            hp, qw = w % 2, w // 2
            rc = drain_sums(w, oacc)
            # both norm halves on DVE: engine-internal ordering is free; a
            # DVE->Pool handoff would sit on the tail critical path
            drain_norm_half(w, oacc, rc, onorm, 0, nc.vector)
            drain_norm_half(w, oacc, rc, onorm, 1, nc.vector)
            tail_sc = [scp.tile([128, 1024], F32, tag="sc", name=f"tsc{i}")
                       for i in range(2)]
            tail_o = opp.tile([128, 1024], F32, tag="oacc", name="tail_o")
            tail_po = [
                tail_sc[0][:, 0:512], tail_o[:, 0:512],
                tail_sc[0][:, 512:1024], tail_o[:, 512:1024],
                tail_sc[1][:, 0:512], tail_sc[0][:, 0:512],
                tail_sc[1][:, 512:1024], tail_sc[0][:, 512:1024],
            ]
            tpt = pop.tile([128, 512], BF, tag="po", name="tp_tail")
            for qsub in range(4):
                for i in range(2):
                    r = 2 * qsub + i
                    on = onorm[r // 4]
                    nc.tensor.transpose(
                        tpt[64 * i:64 * (i + 1), qsub * 128:(qsub + 1) * 128],
                        on[:, (r % 4) * HD:(r % 4 + 1) * HD],
                        ident[:, :],
                        tile_position=(0, 64 * i),
                    )
                col = hp * S + qw * 512 + qsub * 128
                if qsub % 2:
                    nc.vector.tensor_copy(
                        onm2_sb[:, col:col + 128],
                        tpt[:, qsub * 128:(qsub + 1) * 128],
                    )
                else:
                    nc.scalar.copy(
                        onm2_sb[:, col:col + 128],
                        tpt[:, qsub * 128:(qsub + 1) * 128],
                    )
            # all onm2 blocks staged; now the 8 pieces pipeline at copy rate
            for qsub in range(4):
                for n in range(2):
                    outproj_piece(
                        12 + qsub, n, on_act=(n == 0),
                        po=tail_po[2 * qsub + n],
                    )

            if debug:
                nc.sync.dma_start(dbg["qt"][:, :], qt_sb[:, :])
                nc.sync.dma_start(dbg["kt"][:, :], kt_sb[:, :])
                nc.sync.dma_start(dbg["v"][:, :], v_sb[:, :])
                nc.sync.dma_start(dbg["onm2"][:, :], onm2_sb[:, :])

    nc.compile()
    return nc


def _get_nc(with_bias=False):
    if with_bias not in _nc_cache:
        _nc_cache[with_bias] = _build_bass(with_bias=with_bias)
    return _nc_cache[with_bias]


def _prepare_in_maps(x, wq, bq, wk, bk, wv, bv, wo):
    import ml_dtypes

    bf16 = ml_dtypes.bfloat16
    x = np.asarray(x, np.float32)
    wq, bq = np.asarray(wq, np.float32), np.asarray(bq, np.float32)
    wk, bk = np.asarray(wk, np.float32), np.asarray(bk, np.float32)
    wv, bv = np.asarray(wv, np.float32), np.asarray(bv, np.float32)
    wo = np.asarray(wo, np.float32)

    xT = [np.ascontiguousarray(x[b].T).astype(bf16) for b in range(B)]
    in_maps = []
    for c in range(NC):
        b, j = divmod(c, HPC)
        cs = slice(DHC * j, DHC * (j + 1))
        bias3 = np.concatenate([bq[cs], bk[cs], bv[cs]]).reshape(1, 3 * DHC).astype(bf16)
        in_maps.append(
            {
                "xT": xT[b],
                "wq_c": np.ascontiguousarray(wq[:, cs]).astype(bf16),
                "wk_c": np.ascontiguousarray(wk[:, cs]).astype(bf16),
                "wv_c": np.ascontiguousarray(wv[:, cs]).astype(bf16),
                "wo_c": np.ascontiguousarray(wo[cs, :]).astype(bf16),
                "bias3": np.ascontiguousarray(bias3),
            }
        )
    return in_maps


def _gather(parts, bo):
    bo = np.asarray(bo, np.float32)
    out = np.empty((B, S, D), np.float32)
    for b in range(B):
        acc = np.asarray(parts[HPC * b], np.float32)
        for j in range(1, HPC):
            acc = acc + np.asarray(parts[HPC * b + j], np.float32)
        out[b] = acc + bo
    return out


def kernel(x, wq, bq, wk, bk, wv, bv, wo, bo):
    from concourse import bass_utils

    in_maps = _prepare_in_maps(x, wq, bq, wk, bk, wv, bv, wo)
    with_bias = bool(
        np.any(np.asarray(bq)) or np.any(np.asarray(bk)) or np.any(np.asarray(bv))
    )
    res = bass_utils.run_bass_kernel_spmd(
        nc=_get_nc(with_bias), in_maps=in_maps, core_ids=list(range(NC))
    )
    parts = [np.asarray(r["out"], np.float32) for r in res.results]
    return _gather(parts, bo)
